# revision 15
# baseline (speedup 1.0000x reference)
"""2D DCT-II (separable) kernel for Trainium2, data-parallel over 8 NeuronCores.

Problem: img [128, 1, 512, 512] f32 -> out [128, 1, 512, 512] f32 with
    out[b,0] = scale * (C @ img[b,0] @ C^T),  C the scaled DCT-II basis.

Algorithm (v5): the DCT butterfly (reflection fold) on each axis commutes with
the transform on the other axis, so BOTH stages' folds are applied to the raw
image on the host (O(N^2) data prep):
    P = F @ A @ F^T,  F = 2-level butterfly (rows [EE(128); EO(128); O(256)]).
The chip computes two block-matmul stages against three resident basis blocks
(R = F^{-T} C'^T is exactly block-diagonal on CEE/CEO/CO):
    stage1 (data-stationary):  D[nf, pcls] = P[mblk]^T @ {CEE|CEO|CO}
    stage2 (basis-stationary): Yt[qcls, pcls] = {CEE|CEO|CO}^T @ D[nblk]
Stage 2 keeps the tiny basis blocks as the stationary operand (6 LDWEIGHTS +
6 N=512 matmuls per image); its output is Y^T in class order — the host
un-permutes and transposes for free. Everything on chip is bf16 with f32 PSUM
(rel err ~3.4e-3); bf16 I/O keeps HBM traffic at 16 MB/core.

Scheduling: 2-image-deep software pipeline (s1(i+2) emitted before s2(i)) so
the PSUM->SBUF casts have two full s1 windows of slack; [128,1024] 2-bank
paired casts split DVE/ACT (2.4us/engine/image < PE 2.7us); input DMAs on the
GPSIMD SWDGE ring, output DMAs on the ACT HWDGE ring, basis on the SP ring —
each DMA class has a private queue. 1MB paired in/out transfers.
"""

import sys
import numpy as np

for _p in ("/opt/trn_rl_repo", "/opt/pypackages"):
    if _p not in sys.path:
        sys.path.append(_p)

import ml_dtypes  # noqa: E402
import concourse.tile as tile  # noqa: E402
from concourse import bacc, mybir  # noqa: E402
from concourse.bass_utils import run_bass_kernel_spmd  # noqa: E402

N_CORES = 8
B_FULL = 128
S = 512
H = S // 2
Q = S // 4
BPC = B_FULL // N_CORES
T = S // 128
BF16 = ml_dtypes.bfloat16


def _class_order() -> np.ndarray:
    k = np.arange(S)
    return np.concatenate([k[k % 4 == 0], k[k % 4 == 2], k[k % 2 == 1]])


def _butterfly() -> np.ndarray:
    F = np.zeros((S, S))
    E = np.zeros((H, S))
    O = np.zeros((H, S))
    for mp in range(H):
        E[mp, mp] = 1.0
        E[mp, S - 1 - mp] = 1.0
        O[mp, mp] = 1.0
        O[mp, S - 1 - mp] = -1.0
    for mpp in range(Q):
        F[mpp] = E[mpp] + E[H - 1 - mpp]
        F[Q + mpp] = E[mpp] - E[H - 1 - mpp]
    F[H:] = O
    return F


def _basis_fused() -> np.ndarray:
    """[128, 768] bf16: [CEE(128) | CEO(128) | CO[0:128] (256) | CO[128:256] (256)]."""
    j = np.arange(S, dtype=np.float64)
    k = np.arange(S, dtype=np.float64)
    c = np.cos(np.pi * (2.0 * j[:, None] + 1.0) * k[None, :] / (2.0 * S))
    s = np.full(S, np.sqrt(2.0 / S))
    s[0] /= np.sqrt(2.0)
    ct = c * s[None, :]
    R = np.linalg.solve(_butterfly().T, ct)[:, _class_order()]
    cee, ceo, co = R[0:Q, 0:Q], R[Q:H, Q:H], R[H:, H:]
    return np.concatenate([cee, ceo, co[:Q], co[Q:]], axis=1).astype(BF16)


def _prep(img: np.ndarray) -> np.ndarray:
    """Host butterflies: P = F @ A @ F^T, bf16, [B, 512, 512]."""
    A = img.reshape(B_FULL, S, S)
    E = A[:, :H] + A[:, : H - 1 : -1]
    Of = A[:, :H] - A[:, : H - 1 : -1]
    EE = E[:, :Q] + E[:, : Q - 1 : -1]
    EO = E[:, :Q] - E[:, : Q - 1 : -1]
    Pr = np.concatenate([EE, EO, Of], axis=1)
    E2 = Pr[:, :, :H] + Pr[:, :, : H - 1 : -1]
    O2 = Pr[:, :, :H] - Pr[:, :, : H - 1 : -1]
    EE2 = E2[:, :, :Q] + E2[:, :, : Q - 1 : -1]
    EO2 = E2[:, :, :Q] - E2[:, :, : Q - 1 : -1]
    P = np.concatenate([EE2, EO2, O2], axis=2).astype(BF16)
    # pre-tile: [B/2, 2, t, p, n] -> [B/2, p, 2, t, n] (8 KB contiguous/partition)
    return np.ascontiguousarray(
        P.reshape(B_FULL // 2, 2, T, 128, S).transpose(0, 3, 1, 2, 4)
    )


def _build():
    nc = bacc.Bacc("TRN2", target_bir_lowering=False, debug=False)
    # Host-pre-tiled layouts: per pair, each SBUF partition's data is one
    # contiguous 8 KB run in DRAM -> line-rate DMA descriptors.
    p_d = nc.dram_tensor("p", [BPC // 2, 128, 2, T, S], mybir.dt.bfloat16, kind="ExternalInput").ap()
    basis_d = nc.dram_tensor("basis", [128, 6 * Q], mybir.dt.bfloat16, kind="ExternalInput").ap()
    out_d = nc.dram_tensor("out", [BPC // 2, 128, 2, T, S], mybir.dt.bfloat16, kind="ExternalOutput").ap()

    p_v = p_d
    out_v2 = out_d

    with tile.TileContext(nc) as tc:
        with (
            tc.tile_pool(name="const", bufs=1) as cpool,
            tc.tile_pool(name="p", bufs=4) as ppool,
            tc.tile_pool(name="dt", bufs=3) as dtpool,
            tc.tile_pool(name="o", bufs=3) as opool,
            tc.tile_pool(name="ps1", bufs=2, space="PSUM") as ps1pool,
            tc.tile_pool(name="ps2", bufs=4, space="PSUM") as ps2pool,
        ):
            basis_sb = cpool.tile([128, 6 * Q], mybir.dt.bfloat16)
            nc.sync.dma_start(basis_sb[:], basis_d)
            cee = basis_sb[:, 0:Q]
            ceo = basis_sb[:, Q : 2 * Q]
            co0 = basis_sb[:, 2 * Q : 4 * Q]
            co1 = basis_sb[:, 4 * Q : 6 * Q]

            p_tiles = {}

            def emit_load(i2, split=False):
                t = ppool.tile([128, 2, T, S], mybir.dt.bfloat16, tag="p", name=f"p_{i2}")
                if split:
                    nc.gpsimd.dma_start(t[:, 0], p_v[i2, :, 0])
                    nc.gpsimd.dma_start(t[:, 1], p_v[i2, :, 1])
                else:
                    nc.gpsimd.dma_start(t[:], p_v[i2])  # 1 MB, 8 KB/partition contiguous
                p_tiles[i2] = t

            dt_tiles = {}
            o_tiles = {}

            def emit_s1(i):
                p_sb = p_tiles[i // 2]
                tw = i % 2
                if tw == 1:
                    p_tiles.pop(i // 2)
                dt = dtpool.tile([128, T, S], mybir.dt.bfloat16, tag="dt", name=f"dt_{i}")
                for ph in range(2):
                    ps1 = ps1pool.tile([128, 2, S], mybir.dt.float32, tag="ps1", name=f"ps1_{i}_{ph}")
                    for half in range(2):
                        nt = ph * 2 + half
                        ncols = slice(nt * 128, (nt + 1) * 128)
                        nc.tensor.matmul(ps1[:, half, 0:Q], p_sb[:, tw, 0, ncols], cee, start=True, stop=True)
                        nc.tensor.matmul(ps1[:, half, Q:H], p_sb[:, tw, 1, ncols], ceo, start=True, stop=True)
                        nc.tensor.matmul(ps1[:, half, H:S], p_sb[:, tw, 2, ncols], co0, start=True, stop=False)
                        nc.tensor.matmul(ps1[:, half, H:S], p_sb[:, tw, 3, ncols], co1, start=False, stop=True)
                    eng = nc.vector.tensor_copy if ph == 0 else nc.scalar.copy
                    eng(dt[:, 2 * ph : 2 * ph + 2, :], ps1[:])
                dt_tiles[i] = dt

            def emit_s2(i):
                dt = dt_tiles.pop(i)
                tw = i % 2
                last_pair = i >= BPC - 2
                if last_pair:
                    o_sb = opool.tile([128, T, S], mybir.dt.bfloat16, tag="o", name=f"ol_{i}")
                elif tw == 0:
                    o_sb = opool.tile([128, 2, T, S], mybir.dt.bfloat16, tag="o", name=f"o_{i // 2}")
                    o_tiles[i // 2] = o_sb
                else:
                    o_sb = o_tiles.pop(i // 2)
                ov = o_sb if last_pair else o_sb[:, tw]

                ps_q0 = ps2pool.tile([128, S], mybir.dt.float32, tag="ps2", name=f"ps2q0_{i}")
                nc.tensor.matmul(ps_q0[:], cee, dt[:, 0, :], start=True, stop=True)
                nc.scalar.copy(ov[:, 0, :], ps_q0[:])
                ps_q2 = ps2pool.tile([128, S], mybir.dt.float32, tag="ps2", name=f"ps2q2_{i}")
                nc.tensor.matmul(ps_q2[:], ceo, dt[:, 1, :], start=True, stop=True)
                nc.vector.tensor_copy(ov[:, 1, :], ps_q2[:])
                for qc in range(2):
                    qcc = slice(qc * 128, (qc + 1) * 128)
                    ps = ps2pool.tile([128, S], mybir.dt.float32, tag="ps2", name=f"ps2qo_{i}_{qc}")
                    nc.tensor.matmul(ps[:], co0[:, qcc], dt[:, 2, :], start=True, stop=False)
                    nc.tensor.matmul(ps[:], co1[:, qcc], dt[:, 3, :], start=False, stop=True)
                    eng = nc.scalar.copy if qc == 0 else nc.vector.tensor_copy
                    eng(ov[:, 2 + qc, :], ps[:])

                if last_pair:
                    e1 = nc.scalar if tw == 0 else nc.sync
                    e2 = nc.sync if tw == 0 else nc.scalar
                    e1.dma_start(out_v2[i // 2, :, tw, 0:2, :], o_sb[:, 0:2, :])
                    e2.dma_start(out_v2[i // 2, :, tw, 2:4, :], o_sb[:, 2:4, :])
                elif tw == 1:
                    nc.sync.dma_start(out_v2[i // 2], o_sb[:])

            emit_load(0, split=True)
            emit_load(1)
            emit_load(2)
            emit_s1(0)
            emit_s1(1)
            for i in range(BPC):
                if i % 2 == 0 and i // 2 + 3 < BPC // 2:
                    emit_load(i // 2 + 3)
                if i + 2 < BPC:
                    emit_s1(i + 2)
                emit_s2(i)
    nc.compile()
    return nc


_NC_CACHE = None


def _get_nc():
    global _NC_CACHE
    if _NC_CACHE is None:
        _NC_CACHE = _build()
    return _NC_CACHE


def run_sharded(img: np.ndarray, **spmd_kwargs):
    """img [128, 1, 512, 512] f32 -> (out [128, 1, 512, 512] f32, results)."""
    img = np.ascontiguousarray(np.asarray(img, dtype=np.float32))
    P = _prep(img)
    basis = _basis_fused()
    nc = _get_nc()
    hp = BPC // 2
    in_maps = [
        {"p": np.ascontiguousarray(P[k * hp : (k + 1) * hp]), "basis": basis}
        for k in range(N_CORES)
    ]
    res = run_bass_kernel_spmd(nc, in_maps, core_ids=list(range(N_CORES)), **spmd_kwargs)
    raw = np.empty((B_FULL, S, S), dtype=np.float32)
    for k in range(N_CORES):
        # [BPC/2, p, two, c, q] -> [(b2 two), (c p), q]
        r = res.results[k]["out"].astype(np.float32)
        raw[k * BPC : (k + 1) * BPC] = r.transpose(0, 2, 3, 1, 4).reshape(BPC, S, S)
    inv = np.argsort(_class_order())
    out = np.swapaxes(raw[:, inv][:, :, inv], 1, 2)
    return np.ascontiguousarray(out).reshape(B_FULL, 1, S, S), res


def kernel(img: np.ndarray) -> np.ndarray:
    out, _ = run_sharded(img)
    return out


# revision 19
# speedup vs baseline: 1.0413x; 1.0413x over previous
"""2D DCT-II (separable) kernel for Trainium2, data-parallel over 8 NeuronCores.

Problem: img [128, 1, 512, 512] f32 -> out [128, 1, 512, 512] f32 with
    out[b,0] = scale * (C @ img[b,0] @ C^T),  C the scaled DCT-II basis.

Algorithm (v5): the DCT butterfly (reflection fold) on each axis commutes with
the transform on the other axis, so BOTH stages' folds are applied to the raw
image on the host (O(N^2) data prep):
    P = F @ A @ F^T,  F = 2-level butterfly (rows [EE(128); EO(128); O(256)]).
The chip computes two block-matmul stages against three resident basis blocks
(R = F^{-T} C'^T is exactly block-diagonal on CEE/CEO/CO):
    stage1 (data-stationary):  D[nf, pcls] = P[mblk]^T @ {CEE|CEO|CO}
    stage2 (basis-stationary): Yt[qcls, pcls] = {CEE|CEO|CO}^T @ D[nblk]
Stage 2 keeps the tiny basis blocks as the stationary operand (6 LDWEIGHTS +
6 N=512 matmuls per image); its output is Y^T in class order — the host
un-permutes and transposes for free. Everything on chip is bf16 with f32 PSUM
(rel err ~3.4e-3); bf16 I/O keeps HBM traffic at 16 MB/core.

Scheduling: 2-image-deep software pipeline (s1(i+2) emitted before s2(i)) so
the PSUM->SBUF casts have two full s1 windows of slack; [128,1024] 2-bank
paired casts split DVE/ACT (2.4us/engine/image < PE 2.7us); input DMAs on the
GPSIMD SWDGE ring, output DMAs on the ACT HWDGE ring, basis on the SP ring —
each DMA class has a private queue. 1MB paired in/out transfers.
"""

import sys
import numpy as np

for _p in ("/opt/trn_rl_repo", "/opt/pypackages"):
    if _p not in sys.path:
        sys.path.append(_p)

import ml_dtypes  # noqa: E402
import concourse.tile as tile  # noqa: E402
from concourse import bacc, mybir  # noqa: E402
from concourse.bass_utils import run_bass_kernel_spmd  # noqa: E402

N_CORES = 8
B_FULL = 128
S = 512
H = S // 2
Q = S // 4
BPC = B_FULL // N_CORES
T = S // 128
BF16 = ml_dtypes.bfloat16


def _class_order() -> np.ndarray:
    k = np.arange(S)
    return np.concatenate([k[k % 4 == 0], k[k % 4 == 2], k[k % 2 == 1]])


def _butterfly() -> np.ndarray:
    F = np.zeros((S, S))
    E = np.zeros((H, S))
    O = np.zeros((H, S))
    for mp in range(H):
        E[mp, mp] = 1.0
        E[mp, S - 1 - mp] = 1.0
        O[mp, mp] = 1.0
        O[mp, S - 1 - mp] = -1.0
    for mpp in range(Q):
        F[mpp] = E[mpp] + E[H - 1 - mpp]
        F[Q + mpp] = E[mpp] - E[H - 1 - mpp]
    F[H:] = O
    return F


def _basis_fused() -> np.ndarray:
    """[128, 768] bf16: [CEE(128) | CEO(128) | CO[0:128] (256) | CO[128:256] (256)]."""
    j = np.arange(S, dtype=np.float64)
    k = np.arange(S, dtype=np.float64)
    c = np.cos(np.pi * (2.0 * j[:, None] + 1.0) * k[None, :] / (2.0 * S))
    s = np.full(S, np.sqrt(2.0 / S))
    s[0] /= np.sqrt(2.0)
    ct = c * s[None, :]
    R = np.linalg.solve(_butterfly().T, ct)[:, _class_order()]
    cee, ceo, co = R[0:Q, 0:Q], R[Q:H, Q:H], R[H:, H:]
    return np.concatenate([cee, ceo, co[:Q], co[Q:]], axis=1).astype(BF16)


def _prep(img: np.ndarray) -> np.ndarray:
    """Host butterflies: P = F @ A @ F^T, bf16, [B, 512, 512]."""
    A = img.reshape(B_FULL, S, S)
    E = A[:, :H] + A[:, : H - 1 : -1]
    Of = A[:, :H] - A[:, : H - 1 : -1]
    EE = E[:, :Q] + E[:, : Q - 1 : -1]
    EO = E[:, :Q] - E[:, : Q - 1 : -1]
    Pr = np.concatenate([EE, EO, Of], axis=1)
    E2 = Pr[:, :, :H] + Pr[:, :, : H - 1 : -1]
    O2 = Pr[:, :, :H] - Pr[:, :, : H - 1 : -1]
    EE2 = E2[:, :, :Q] + E2[:, :, : Q - 1 : -1]
    EO2 = E2[:, :, :Q] - E2[:, :, : Q - 1 : -1]
    P = np.concatenate([EE2, EO2, O2], axis=2).astype(BF16)
    # pre-tile: [B/2, 2, t, p, n] -> [B/2, p, 2, t, n] (8 KB contiguous/partition)
    return np.ascontiguousarray(
        P.reshape(B_FULL // 2, 2, T, 128, S).transpose(0, 3, 1, 2, 4)
    )


def _build():
    nc = bacc.Bacc("TRN2", target_bir_lowering=False, debug=False)
    # Host-pre-tiled layouts: per pair, each SBUF partition's data is one
    # contiguous 8 KB run in DRAM -> line-rate DMA descriptors.
    p_d = nc.dram_tensor("p", [BPC // 2, 128, 2, T, S], mybir.dt.bfloat16, kind="ExternalInput").ap()
    basis_d = nc.dram_tensor("basis", [128, 6 * Q], mybir.dt.bfloat16, kind="ExternalInput").ap()
    out_d = nc.dram_tensor("out", [BPC // 2, 128, 2, T, S], mybir.dt.bfloat16, kind="ExternalOutput").ap()

    p_v = p_d
    out_v2 = out_d

    with tile.TileContext(nc) as tc:
        with (
            tc.tile_pool(name="const", bufs=1) as cpool,
            tc.tile_pool(name="p", bufs=5) as ppool,
            tc.tile_pool(name="dt", bufs=4) as dtpool,
            tc.tile_pool(name="o", bufs=3) as opool,
            tc.tile_pool(name="ps1", bufs=2, space="PSUM") as ps1pool,
            tc.tile_pool(name="ps2", bufs=2, space="PSUM") as ps2pool,
        ):
            basis_sb = cpool.tile([128, 6 * Q], mybir.dt.bfloat16)
            nc.sync.dma_start(basis_sb[:], basis_d)
            cee = basis_sb[:, 0:Q]
            ceo = basis_sb[:, Q : 2 * Q]
            co0 = basis_sb[:, 2 * Q : 4 * Q]
            co1 = basis_sb[:, 4 * Q : 6 * Q]

            p_tiles = {}

            def emit_load(i2, split=False):
                t = ppool.tile([128, 2, T, S], mybir.dt.bfloat16, tag="p", name=f"p_{i2}")
                if split:
                    nc.sync.dma_start(t[:, 0], p_v[i2, :, 0])
                    nc.sync.dma_start(t[:, 1], p_v[i2, :, 1])
                else:
                    nc.sync.dma_start(t[:], p_v[i2])  # 1 MB, 8 KB/partition contiguous
                p_tiles[i2] = t

            dt_tiles = {}
            o_tiles = {}

            def emit_s1(i):
                p_sb = p_tiles[i // 2]
                tw = i % 2
                if tw == 1:
                    p_tiles.pop(i // 2)
                dt = dtpool.tile([128, T, S], mybir.dt.bfloat16, tag="dt", name=f"dt_{i}")
                for ph in range(2):
                    ps1 = ps1pool.tile([128, 2, S], mybir.dt.float32, tag="ps1", name=f"ps1_{i}_{ph}")
                    for half in range(2):
                        nt = ph * 2 + half
                        ncols = slice(nt * 128, (nt + 1) * 128)
                        nc.tensor.matmul(ps1[:, half, 0:Q], p_sb[:, tw, 0, ncols], cee, start=True, stop=True)
                        nc.tensor.matmul(ps1[:, half, Q:H], p_sb[:, tw, 1, ncols], ceo, start=True, stop=True)
                        nc.tensor.matmul(ps1[:, half, H:S], p_sb[:, tw, 2, ncols], co0, start=True, stop=False)
                        nc.tensor.matmul(ps1[:, half, H:S], p_sb[:, tw, 3, ncols], co1, start=False, stop=True)
                    eng = nc.vector.tensor_copy if ph == 0 else nc.scalar.copy
                    eng(dt[:, 2 * ph : 2 * ph + 2, :], ps1[:])
                dt_tiles[i] = dt

            def emit_s2(i):
                dt = dt_tiles.pop(i)
                tw = i % 2
                last_pair = i >= BPC - 2
                if last_pair:
                    o_sb = opool.tile([128, T, S], mybir.dt.bfloat16, tag="o", name=f"ol_{i}")
                elif tw == 0:
                    o_sb = opool.tile([128, 2, T, S], mybir.dt.bfloat16, tag="o", name=f"o_{i // 2}")
                    o_tiles[i // 2] = o_sb
                else:
                    o_sb = o_tiles.pop(i // 2)
                ov = o_sb if last_pair else o_sb[:, tw]

                psA = ps2pool.tile([128, 2, S], mybir.dt.float32, tag="ps2", name=f"ps2a_{i}")
                nc.tensor.matmul(psA[:, 0, :], cee, dt[:, 0, :], start=True, stop=True)
                nc.tensor.matmul(psA[:, 1, :], ceo, dt[:, 1, :], start=True, stop=True)
                nc.scalar.copy(ov[:, 0:2, :], psA[:])
                psB = ps2pool.tile([128, 2, S], mybir.dt.float32, tag="ps2", name=f"ps2b_{i}")
                for qc in range(2):
                    qcc = slice(qc * 128, (qc + 1) * 128)
                    nc.tensor.matmul(psB[:, qc, :], co0[:, qcc], dt[:, 2, :], start=True, stop=False)
                    nc.tensor.matmul(psB[:, qc, :], co1[:, qcc], dt[:, 3, :], start=False, stop=True)
                nc.vector.tensor_copy(ov[:, 2:4, :], psB[:])

                if last_pair:
                    e1 = nc.scalar if tw == 0 else nc.sync
                    e2 = nc.sync if tw == 0 else nc.scalar
                    e1.dma_start(out_v2[i // 2, :, tw, 0:2, :], o_sb[:, 0:2, :])
                    e2.dma_start(out_v2[i // 2, :, tw, 2:4, :], o_sb[:, 2:4, :])
                elif tw == 1:
                    nc.scalar.dma_start(out_v2[i // 2], o_sb[:])

            emit_load(0, split=True)
            emit_load(1)
            emit_load(2)
            emit_s1(0)
            emit_s1(1)
            emit_s1(2)
            for i in range(BPC):
                if i % 2 == 0 and i // 2 + 3 < BPC // 2:
                    emit_load(i // 2 + 3)
                if i + 3 < BPC:
                    emit_s1(i + 3)
                emit_s2(i)
    nc.compile()
    return nc


_NC_CACHE = None


def _get_nc():
    global _NC_CACHE
    if _NC_CACHE is None:
        _NC_CACHE = _build()
    return _NC_CACHE


def run_sharded(img: np.ndarray, **spmd_kwargs):
    """img [128, 1, 512, 512] f32 -> (out [128, 1, 512, 512] f32, results)."""
    img = np.ascontiguousarray(np.asarray(img, dtype=np.float32))
    P = _prep(img)
    basis = _basis_fused()
    nc = _get_nc()
    hp = BPC // 2
    in_maps = [
        {"p": np.ascontiguousarray(P[k * hp : (k + 1) * hp]), "basis": basis}
        for k in range(N_CORES)
    ]
    res = run_bass_kernel_spmd(nc, in_maps, core_ids=list(range(N_CORES)), **spmd_kwargs)
    raw = np.empty((B_FULL, S, S), dtype=np.float32)
    for k in range(N_CORES):
        # [BPC/2, p, two, c, q] -> [(b2 two), (c p), q]
        r = res.results[k]["out"].astype(np.float32)
        raw[k * BPC : (k + 1) * BPC] = r.transpose(0, 2, 3, 1, 4).reshape(BPC, S, S)
    inv = np.argsort(_class_order())
    out = np.swapaxes(raw[:, inv][:, :, inv], 1, 2)
    return np.ascontiguousarray(out).reshape(B_FULL, 1, S, S), res


def kernel(img: np.ndarray) -> np.ndarray:
    out, _ = run_sharded(img)
    return out
